# revision 1
# baseline (speedup 1.0000x reference)
"""ChannelAttention (CBAM-style) Trainium2 Bass kernel — channel-major bf16.

Reference computation (per batch image):
    avg = mean(x, spatial)             # [C]
    mx  = max(x, spatial)              # [C]
    s   = sigmoid(mlp(avg) + mlp(max)) # mlp: relu(p@w1+b1)@w2+b2
    y   = x * s[None, None, :]

Full shapes: x [32, 112, 112, 256] f32.  Data-parallel over batch: each of
the 8 NeuronCores handles 4 images; the tiny MLP weights are replicated.

Layout: the host transposes x to channel-major [img, C, HW] and converts to
bf16 (the correctness gate is 2e-2 max relative error; bf16 is ~2e-3).
This halves HBM traffic AND puts channels on SBUF partitions, so:
  - sum-pool rides an in-place tensor_scalar(x*1) with accum_out, which the
    DVE runs in 4x perf mode on packed bf16 (0.30 ns/el),
  - max-pool is a tensor_tensor(max) fold tree on DVE (2x mode, 0.52 ns/el),
  - the scale multiply becomes activation(Copy, scale=[P,1]) on ACT --
    per-partition scale -- freeing the DVE for the pooling of the next
    image (ACT: 0.833 ns/el),
  - the MLP runs column-major ([C->partitions] vectors) on PE + ACT.
DMA: loads and stores are [128, 6272] bf16 transfers (12544B/partition
contiguous) round-robined over all three DMA queues (SP + ACT HWDGE rings
and the Pool SWDGE ring), which the cost model overlaps fully: each queue
sustains 360 B/ns.  Per core ~51.4MB total traffic / 3 queues ~= 48us,
under the ~22us/img engine pipeline (DVE 21.8 / ACT 21.4).
"""

import sys

import numpy as np

for _p in ("/opt/trn_rl_repo",):
    if _p not in sys.path:
        sys.path.append(_p)

import concourse.bass as bass
import concourse.tile as tile
from concourse import mybir

B, HWs, C = 32, 112 * 112, 256  # 12544 spatial positions per image
HID = 32
N_CORES = 8
IMG = B // N_CORES  # 4 images per core
P = 128
NBLK = C // P  # 2 channel blocks
HALF = HWs // 2  # 6272: load/store/pool tile width
F32 = mybir.dt.float32
F16 = mybir.dt.float16
BF16 = mybir.dt.bfloat16


def _split_multiwait(nc, max_waits=1):
    """This walrus build rejects >1 sync wait per instruction.  Hoist
    excess waits onto InstNoOp instructions inserted just before, on the
    same engine (same semantics: the sequencer blocks on each in turn)."""
    for f in nc.m.functions:
        for bb in f.blocks:
            new_insts = []
            for ins in bb.instructions:
                si = ins.sync_info
                w = list(si.on_wait) if si and si.on_wait else []
                if len(w) > max_waits:
                    for j, ww in enumerate(w[:-max_waits]):
                        nop = mybir.InstNoOp(
                            name=f"{ins.name}.sw{j}",
                            engine=ins.engine,
                            sync_info=mybir.SyncInfo(on_wait=[ww], on_update=[]),
                        )
                        nc.register_instruction(nop, overwrite=True)
                        new_insts.append(nop)
                    si.on_wait = w[-max_waits:]
                new_insts.append(ins)
            bb.instructions = new_insts
    return nc


def build_nc(n_img=IMG, xbufs=3):
    nc = bass.Bass()
    rows = n_img * NBLK * P
    x_d = nc.declare_dram_parameter("x", [rows, HWs], F16, isOutput=False)
    w1_d = nc.declare_dram_parameter("w1", [C, HID], F32, isOutput=False)
    b1_d = nc.declare_dram_parameter("b1", [HID], F32, isOutput=False)
    w2_d = nc.declare_dram_parameter("w2", [HID, C], F32, isOutput=False)
    b2c_d = nc.declare_dram_parameter("b2c", [P, NBLK], F32, isOutput=False)
    # y is bf16: fp16 subnormals below ~6e-5 would quantize tiny outputs to
    # a 6e-8 absolute grid (3% of the checker's 1e-6 rel-err floor); bf16
    # keeps uniform 2^-9 relative precision at every magnitude.
    y_d = nc.declare_dram_parameter("y", [rows, HWs], BF16, isOutput=True)

    xv = x_d.rearrange("(i b p) s -> i b p s", i=n_img, b=NBLK)
    yv = y_d.rearrange("(i b p) s -> i b p s", i=n_img, b=NBLK)

    AF = mybir.ActivationFunctionType
    OP = mybir.AluOpType
    AX = mybir.AxisListType

    with tile.TileContext(nc) as tc:
        with (
            tc.tile_pool(name="singles", bufs=1) as singles,
            tc.tile_pool(name="xb", bufs=xbufs) as xb_pool,
            tc.tile_pool(name="macc", bufs=2) as macc_pool,
            tc.tile_pool(name="small", bufs=3) as small,
            tc.tile_pool(name="ps_small", bufs=4, space="PSUM") as ps_small,
        ):
            # --- replicated constants ---
            w1_sb = singles.tile([P, NBLK, HID], F32)
            nc.sync.dma_start(out=w1_sb[:], in_=w1_d.rearrange("(b p) h -> p b h", b=NBLK))
            w2_sb = singles.tile([HID, NBLK, P], F32)
            nc.sync.dma_start(out=w2_sb[:], in_=w2_d.rearrange("h (b p) -> h b p", b=NBLK))
            b1_sb = singles.tile([HID, 1], F32)
            nc.sync.dma_start(out=b1_sb[:], in_=b1_d.rearrange("(p o) -> p o", o=1))
            b2c_sb = singles.tile([P, NBLK], F32)
            nc.sync.dma_start(out=b2c_sb[:], in_=b2c_d[:, :])

            # DMA queue plan per image: b0's halves land first (two parallel
            # queues) so pooling starts ASAP; the ACT ring gets exactly one
            # load per image (each HWDGE issue holds the issuing SEQ ~1.3us,
            # and ACT's SEQ must stay free to feed the multiplies).
            LOADQ = ["sync", "gpsimd", "sync", "gpsimd"]
            STOREQ = ["sync", "gpsimd", "sync", "gpsimd"]
            q = lambda nm: {"sync": nc.sync, "scalar": nc.scalar,
                            "gpsimd": nc.gpsimd}[nm]

            def issue_loads(img, quarters=False):
                t = [xb_pool.tile([P, HWs], F16, tag=f"xb{blk}",
                                  name=f"xb{blk}_{img}")
                     for blk in range(NBLK)]
                if quarters:
                    # pipeline fill: split the first image's loads so the
                    # first pool op starts after a quarter-tile, not a half
                    qq = ["sync", "gpsimd"] * 4 + ["scalar", "sync", "scalar",
                                                   "gpsimd"]
                    QU = HWs // 4
                    for k, (blk, h) in enumerate(
                        [(b, j) for b in range(NBLK) for j in range(4)]
                    ):
                        sl = slice(h * QU, (h + 1) * QU)
                        q(qq[k]).dma_start(out=t[blk][:, sl], in_=xv[img, blk][:, sl])
                    return t
                for k, (blk, h) in enumerate(((0, 0), (0, 1), (1, 0), (1, 1))):
                    sl = slice(h * HALF, (h + 1) * HALF)
                    q(LOADQ[k]).dma_start(out=t[blk][:, sl], in_=xv[img, blk][:, sl])
                return t

            xb_next = issue_loads(0)
            QU = HWs // 4
            DQ = slice(3 * QU, HWs)  # quarter whose multiply rides the DVE
            deferred = None  # (img, xb1_tile, s_cols) from the previous image

            for img in range(n_img):
                xb = xb_next
                if img + 1 < n_img:
                    # hoist next image's loads ahead of this image's
                    # mult+stores so they never queue behind them
                    xb_next = issue_loads(img + 1)

                # --- pooling (DVE) ---
                # sum: in-place x*1.0 with accum_out; 4x perf mode on fp16
                sums = small.tile([P, NBLK], F32, tag="sums")
                pmax = small.tile([P, NBLK], F32, tag="pmax")
                for blk in range(NBLK):
                    if img == 0 and blk == 1:
                        # fill: ACT is idle during the first pool; a Copy
                        # with accum_out (main out discarded into a stride-0
                        # dummy so the DVE max isn't serialized behind a
                        # full-tile write) shortens the first pool by ~5us
                        dummy = small.tile([P, 1], F16, tag="dummy")
                        nc.scalar.activation(
                            out=dummy[:, :].broadcast_to((P, HWs)),
                            in_=xb[blk][:, :], func=AF.Copy,
                            accum_out=sums[:, blk : blk + 1],
                        )
                    else:
                        nc.vector.tensor_scalar(
                            out=xb[blk][:, :], in0=xb[blk][:, :],
                            scalar1=1.0, scalar2=0.0,
                            op0=OP.mult, op1=OP.add,
                            accum_out=sums[:, blk : blk + 1],
                        )
                    # max: fold tree at 2x
                    macc = macc_pool.tile([P, HALF], F16, tag="macc")
                    nc.vector.tensor_tensor(
                        out=macc[:], in0=xb[blk][:, 0:HALF],
                        in1=xb[blk][:, HALF:HWs], op=OP.max,
                    )
                    w = HALF // 2
                    while w >= 196:  # 3136 ... 196; then one 1x reduce
                        nc.vector.tensor_tensor(
                            out=macc[:, 0:w], in0=macc[:, 0:w],
                            in1=macc[:, w : 2 * w], op=OP.max,
                        )
                        w //= 2
                    nc.vector.reduce_max(
                        out=pmax[:, blk : blk + 1], in_=macc[:, 0:196], axis=AX.X
                    )

                if deferred is not None:
                    # previous image's deferred quarter-multiply, placed
                    # after this pool in the DVE stream (its sigmoid is long
                    # ready, so the DVE never stalls on it)
                    pimg, pt, psc = deferred
                    deferred = None
                    nc.vector.tensor_scalar_mul(
                        out=pt[:, DQ].bitcast(BF16), in0=pt[:, DQ],
                        scalar1=psc[:, 1:2],
                    )
                    q("gpsimd").dma_start(
                        out=yv[pimg, 1][:, DQ], in_=pt[:, DQ].bitcast(BF16)
                    )

                # --- MLP (PE + ACT), column-major ---
                avg2 = sums
                h_sb = small.tile([HID, 2], F32, tag="h")
                for j, (pc, sc) in enumerate(((avg2, 1.0 / HWs), (pmax, 1.0))):
                    ph = ps_small.tile([HID, 1], F32, tag="ph")
                    nc.tensor.matmul(
                        ph[:], lhsT=w1_sb[:, 0, :], rhs=pc[:, 0:1],
                        start=True, stop=False,
                    )
                    nc.tensor.matmul(
                        ph[:], lhsT=w1_sb[:, 1, :], rhs=pc[:, 1:2],
                        start=False, stop=True,
                    )
                    nc.scalar.activation(
                        out=h_sb[:, j : j + 1], in_=ph[:], func=AF.Relu,
                        bias=b1_sb[:], scale=sc,
                    )
                s_cols = small.tile([P, NBLK], F32, tag="scol")
                for blk in range(NBLK):
                    psy = ps_small.tile([P, 1], F32, tag="psy")
                    nc.tensor.matmul(
                        psy[:], lhsT=w2_sb[:, blk, :], rhs=h_sb[:, 0:1],
                        start=True, stop=False,
                    )
                    nc.tensor.matmul(
                        psy[:], lhsT=w2_sb[:, blk, :], rhs=h_sb[:, 1:2],
                        start=False, stop=True,
                    )
                    nc.scalar.activation(
                        out=s_cols[:, blk : blk + 1], in_=psy[:],
                        func=AF.Sigmoid, bias=b2c_sb[:, blk : blk + 1], scale=1.0,
                    )

                # --- scale (ACT: per-partition scale) + store ---
                # --- scale + store ---
                # Tail handling: the last image's multiplies run on the DVE
                # (its 4x tensor_scalar is ~3x faster than ACT and the DVE is
                # idle after the final pool); the second-to-last image defers
                # one block to the DVE so ACT frees up early for the last
                # image's MLP instead of pinning it behind a 21us multiply.
                if img == n_img - 1:
                    # drain: quarter-tile multiplies, 6 on the DVE's 4x
                    # tensor_scalar and 2 on ACT in parallel, each store
                    # firing immediately on a rotating queue, so the tail
                    # after the last pool is ~6us of mult + one store
                    QU = HWs // 4
                    sq = ["sync", "gpsimd", "scalar"]
                    quarters = [(b, j) for b in range(NBLK) for j in range(4)]
                    for k, (blk, j) in enumerate(quarters):
                        sl = slice(j * QU, (j + 1) * QU)
                        if blk == 1 and j >= 2:  # last two quarters on ACT
                            nc.scalar.activation(
                                out=xb[blk][:, sl].bitcast(BF16),
                                in_=xb[blk][:, sl], func=AF.Copy,
                                scale=s_cols[:, blk : blk + 1],
                            )
                        else:
                            nc.vector.tensor_scalar_mul(
                                out=xb[blk][:, sl].bitcast(BF16),
                                in0=xb[blk][:, sl],
                                scalar1=s_cols[:, blk : blk + 1],
                            )
                        q(sq[k % 3]).dma_start(
                            out=yv[img, blk][:, sl], in_=xb[blk][:, sl].bitcast(BF16)
                        )
                    continue
                # in-place multiply, rewriting the fp16 tile as bf16.  ACT
                # carries 7/8 of it; one quarter-tile rides the DVE's 4x
                # tensor_scalar to even out the two engines' per-image load,
                # deferred until after the NEXT image's pool so the DVE never
                # stalls waiting for this image's sigmoid.
                QU = HWs // 4
                deferred = (img, xb[1], s_cols)
                nc.scalar.activation(
                    out=xb[0][:, :].bitcast(BF16), in_=xb[0][:, :],
                    func=AF.Copy, scale=s_cols[:, 0:1],
                )
                nc.scalar.activation(
                    out=xb[1][:, 0 : 3 * QU].bitcast(BF16),
                    in_=xb[1][:, 0 : 3 * QU],
                    func=AF.Copy, scale=s_cols[:, 1:2],
                )
                for blk in range(NBLK):
                    for h in range(2):
                        if blk == 1 and h == 1:
                            continue  # stored by the deferred quarter-mult
                        sl = slice(h * HALF, (h + 1) * HALF)
                        q(STOREQ[blk * 2 + h]).dma_start(
                            out=yv[img, blk][:, sl],
                            in_=xb[blk][:, sl].bitcast(BF16),
                        )
                q(STOREQ[3]).dma_start(
                    out=yv[img, 1][:, 2 * QU : 3 * QU],
                    in_=xb[1][:, 2 * QU : 3 * QU].bitcast(BF16),
                )

    _split_multiwait(nc)
    return nc


# ---------------------------------------------------------------------------
# host-side driver
# ---------------------------------------------------------------------------

_CACHED = {}


def _get_nc():
    if "nc" not in _CACHED:
        _CACHED["nc"] = build_nc()
    return _CACHED["nc"]


def kernel(x, w1, b1, w2, b2):
    
    from concourse.bass_utils import run_bass_kernel_spmd

    f16 = np.float16
    x = np.asarray(x, dtype=np.float32)
    assert x.shape == (B, 112, 112, C)
    xr = x.reshape(B, HWs, C)
    w1 = np.ascontiguousarray(w1, dtype=np.float32)
    b1 = np.ascontiguousarray(b1, dtype=np.float32)
    w2 = np.ascontiguousarray(w2, dtype=np.float32)
    b2c = np.ascontiguousarray(
        (2.0 * np.asarray(b2, dtype=np.float32)).reshape(NBLK, P).T
    )
    in_maps = []
    for c in range(N_CORES):
        xc = xr[c * IMG : (c + 1) * IMG]  # [4, 12544, 256]
        xc = np.ascontiguousarray(xc.transpose(0, 2, 1)).astype(f16)
        in_maps.append(
            {
                "x": xc.reshape(IMG * NBLK * P, HWs),
                "w1": w1,
                "b1": b1,
                "w2": w2,
                "b2c": b2c,
            }
        )
    nc = _get_nc()
    res = run_bass_kernel_spmd(nc, in_maps, list(range(N_CORES)))
    out = np.empty((B, C, HWs), dtype=np.float32)
    for c in range(N_CORES):
        out[c * IMG : (c + 1) * IMG] = (
            res.results[c]["y"].reshape(IMG, C, HWs).astype(np.float32)
        )
    return np.ascontiguousarray(out.transpose(0, 2, 1)).reshape(B, 112, 112, C)



# revision 17
# speedup vs baseline: 1.2095x; 1.2095x over previous
"""ChannelAttention (CBAM-style) Trainium2 Bass kernel.

Reference computation (per batch image):
    avg = mean(x, spatial)             # [C]
    mx  = max(x, spatial)              # [C]
    s   = sigmoid(mlp(avg) + mlp(max)) # mlp: relu(p@w1+b1)@w2+b2
    y   = x * s[None, None, :]

Full shapes: x [32, 112, 112, 256] f32.  Data-parallel over batch: each of
the 8 NeuronCores handles 4 images; the tiny MLP weights are replicated.

Per-core structure (4 images, channel-major fp16 [C=2x128p, HW=12544]):
  - Pooling: BOTH stats ride the DVE's 4x-perf-mode tensor_scalar(x*1)
    accumulator: op1=add gives the free-axis sum, op1=max the free-axis
    max (0.26 ns/el) -- one instruction per stat per channel block, no
    fold trees.  13.1us/img on DVE.
  - MLP on PE (fp32 matmuls, column-major) + ACT (Relu/Sigmoid share one
    activation table with Copy, so no table reloads).
  - Scale multiply in-place (fp16 tile rewritten as bf16): DVE
    tensor_scalar_mul for cols [0,M1) at 0.26 ns/el, ACT Copy-with-scale
    for [M1,HW) at 0.83 ns/el, sized so both engines finish together.
    The DVE chunk for image i is emitted after image i+1's pooling so the
    DVE never stalls waiting on sigmoid_i.
  - DMA: quarter-block (3136-col) transfers, 2.42us each in the cost
    model, spread over the three DMA queues (SP and ACT HWDGE, Pool
    SWDGE) so every queue carries ~16.7us/img; loads prefetch one image
    ahead, stores fire per quarter as soon as its multiply chunk lands.
Every track (SP, Pool, ACT, DVE) carries ~16.7us/img; makespan is
4 x 16.7us plus pipeline fill/drain.
"""

import sys

import numpy as np

for _p in ("/opt/trn_rl_repo",):
    if _p not in sys.path:
        sys.path.append(_p)

import concourse.bass as bass
import concourse.tile as tile
from concourse import mybir

B, HWs, C = 32, 112 * 112, 256
HID = 32
N_CORES = 8
IMG = B // N_CORES  # 4 images per core
P = 128
NBLK = C // P  # 2 channel blocks
QU = HWs // 4  # 3136-col DMA quarter
F32 = mybir.dt.float32
F16 = mybir.dt.float16
BF16 = mybir.dt.bfloat16

# multiply column split: DVE does [0, M1), ACT does [M1, HWs) per block
M1 = 6370


def _split_multiwait(nc, max_waits=1):
    """This walrus build rejects >1 sync wait per instruction.  Hoist
    excess waits onto InstNoOp instructions inserted just before, on the
    same engine (same semantics: the sequencer blocks on each in turn)."""
    for f in nc.m.functions:
        for bb in f.blocks:
            new_insts = []
            for ins in bb.instructions:
                si = ins.sync_info
                w = list(si.on_wait) if si and si.on_wait else []
                if len(w) > max_waits:
                    for j, ww in enumerate(w[:-max_waits]):
                        nop = mybir.InstNoOp(
                            name=f"{ins.name}.sw{j}",
                            engine=ins.engine,
                            sync_info=mybir.SyncInfo(on_wait=[ww], on_update=[]),
                        )
                        nc.register_instruction(nop, overwrite=True)
                        new_insts.append(nop)
                    si.on_wait = w[-max_waits:]
                new_insts.append(ins)
            bb.instructions = new_insts
    return nc


def build_nc(n_img=IMG):
    nc = bass.Bass()
    x_d = nc.declare_dram_parameter("x", [n_img * C, HWs], F16, isOutput=False)
    w1_d = nc.declare_dram_parameter("w1", [C, HID], F32, isOutput=False)
    b1_d = nc.declare_dram_parameter("b1", [HID], F32, isOutput=False)
    w2_d = nc.declare_dram_parameter("w2", [HID, C], F32, isOutput=False)
    b2c_d = nc.declare_dram_parameter("b2c", [P, NBLK], F32, isOutput=False)
    y_d = nc.declare_dram_parameter("y", [n_img * C, HWs], BF16, isOutput=True)

    xv = x_d.rearrange("(i b p) s -> i b p s", i=n_img, b=NBLK)
    yv = y_d.rearrange("(i b p) s -> i b p s", i=n_img, b=NBLK)

    AF = mybir.ActivationFunctionType
    OP = mybir.AluOpType

    with tile.TileContext(nc) as tc:
        with (
            tc.tile_pool(name="singles", bufs=1) as singles,
            tc.tile_pool(name="xb", bufs=4) as xb_pool,
            tc.tile_pool(name="small", bufs=3) as small,
            tc.tile_pool(name="ps_small", bufs=4, space="PSUM") as ps_small,
        ):
            # --- replicated constants, declared up front, loaded after the
            # first image's quarters are in flight (MLP needs them ~15us in)
            w1_sb = singles.tile([P, NBLK, HID], F32)
            w2_sb = singles.tile([HID, NBLK, P], F32)
            b1_sb = singles.tile([HID, 1], F32)
            b2c_sb = singles.tile([P, NBLK], F32)

            def load_consts():
                nc.scalar.dma_start(out=w1_sb[:], in_=w1_d.rearrange("(b p) h -> p b h", b=NBLK))
                nc.scalar.dma_start(out=w2_sb[:], in_=w2_d.rearrange("h (b p) -> h b p", b=NBLK))
                nc.scalar.dma_start(out=b1_sb[:], in_=b1_d.rearrange("(p o) -> p o", o=1))
                nc.scalar.dma_start(out=b2c_sb[:], in_=b2c_d[:, :])

            # quarter-granular DMA queue assignment (per image):
            # loads: SP x3, Pool x4, ACT x1; stores: SP x4, Pool x3, ACT x1
            LOADQ = ["sync", "gpsimd", "scalar", "sync",
                     "gpsimd", "sync", "gpsimd", "gpsimd"]
            STOREQ = ["sync", "gpsimd", "sync", "scalar",
                      "gpsimd", "sync", "gpsimd", "sync"]
            q = lambda nm: {"sync": nc.sync, "scalar": nc.scalar,
                            "gpsimd": nc.gpsimd}[nm]

            def issue_loads(img):
                t = [
                    xb_pool.tile([P, HWs], F16, tag=f"xb{blk}", name=f"xb{blk}_{img}")
                    for blk in range(NBLK)
                ]
                if img == 0:
                    # fill: block 0's quarters spread over all three queues so
                    # the first pooling instruction starts ~5us sooner
                    qs = ["sync", "gpsimd", "scalar", "sync",
                          "gpsimd", "scalar", "gpsimd", "sync"]
                else:
                    qs = LOADQ
                for k, (blk, j) in enumerate(
                    [(b, j) for b in range(NBLK) for j in range(4)]
                ):
                    sl = slice(j * QU, (j + 1) * QU)
                    q(qs[k]).dma_start(out=t[blk][:, sl], in_=xv[img, blk][:, sl])
                return t

            def do_mul_stores(img, xb, s_cols, tail=False):
                """Multiply chunks + their stores for `img`.  The DVE chunk
                covers [0, m1); ACT covers [m1, HWs).  On the tail image the
                split leans DVE-ward and muls/stores interleave per block so
                the drain is as short as possible."""
                m1 = 3 * QU if tail else M1
                # tail store queues chosen so each queue gets at most 2 of
                # the final quarters and block 1's land on distinct queues
                tailq = ["sync", "gpsimd", "scalar", "sync",
                         "gpsimd", "scalar", "sync", "gpsimd"]
                for blk in range(NBLK):
                    nc.vector.tensor_scalar_mul(
                        out=xb[blk][:, 0:m1].bitcast(BF16),
                        in0=xb[blk][:, 0:m1],
                        scalar1=s_cols[:, blk : blk + 1],
                    )
                    nc.scalar.activation(
                        out=xb[blk][:, m1:HWs].bitcast(BF16),
                        in_=xb[blk][:, m1:HWs], func=AF.Copy,
                        scale=s_cols[:, blk : blk + 1],
                    )
                    if tail:
                        for j in range(4):
                            sl = slice(j * QU, (j + 1) * QU)
                            q(tailq[blk * 4 + j]).dma_start(
                                out=yv[img, blk][:, sl],
                                in_=xb[blk][:, sl].bitcast(BF16),
                            )
                if not tail:
                    for k, (blk, j) in enumerate(
                        [(b, j) for b in range(NBLK) for j in range(4)]
                    ):
                        sl = slice(j * QU, (j + 1) * QU)
                        q(STOREQ[k]).dma_start(
                            out=yv[img, blk][:, sl], in_=xb[blk][:, sl].bitcast(BF16)
                        )

            xb_next = issue_loads(0)
            deferred = None  # (img, xb, s_cols) awaiting DVE mul + stores

            for img in range(n_img):
                xb = xb_next
                if img + 1 < n_img:
                    xb_next = issue_loads(img + 1)
                if img == 0:
                    load_consts()

                # --- pooling (DVE 4x accumulate: op1=add -> sum, op1=max) ---
                sums = small.tile([P, NBLK], F32, tag="sums")
                pmax = small.tile([P, NBLK], F32, tag="pmax")
                for blk in range(NBLK):
                    nc.vector.tensor_scalar(
                        out=xb[blk][:, :], in0=xb[blk][:, :],
                        scalar1=1.0, scalar2=0.0, op0=OP.mult, op1=OP.add,
                        accum_out=sums[:, blk : blk + 1],
                    )
                    nc.vector.tensor_scalar(
                        out=xb[blk][:, :], in0=xb[blk][:, :],
                        scalar1=1.0, scalar2=None, op0=OP.mult, op1=OP.max,
                        accum_out=pmax[:, blk : blk + 1],
                    )

                # previous image's deferred DVE mul chunk + all its stores:
                # placed after this image's pooling so the DVE never waits
                # on the (long ready) previous sigmoid.
                if deferred is not None:
                    pimg, pxb, psc = deferred
                    deferred = None
                    do_mul_stores(pimg, pxb, psc)

                # --- MLP (PE + ACT), column-major ---
                h_sb = small.tile([HID, 2], F32, tag="h")
                for j, (pc, sc) in enumerate(((sums, 1.0 / HWs), (pmax, 1.0))):
                    ph = ps_small.tile([HID, 1], F32, tag="ph")
                    nc.tensor.matmul(
                        ph[:], lhsT=w1_sb[:, 0, :], rhs=pc[:, 0:1],
                        start=True, stop=False,
                    )
                    nc.tensor.matmul(
                        ph[:], lhsT=w1_sb[:, 1, :], rhs=pc[:, 1:2],
                        start=False, stop=True,
                    )
                    nc.scalar.activation(
                        out=h_sb[:, j : j + 1], in_=ph[:], func=AF.Relu,
                        bias=b1_sb[:], scale=sc,
                    )
                s_cols = small.tile([P, NBLK], F32, tag="scol")
                for blk in range(NBLK):
                    psy = ps_small.tile([P, 1], F32, tag="psy")
                    nc.tensor.matmul(
                        psy[:], lhsT=w2_sb[:, blk, :], rhs=h_sb[:, 0:1],
                        start=True, stop=False,
                    )
                    nc.tensor.matmul(
                        psy[:], lhsT=w2_sb[:, blk, :], rhs=h_sb[:, 1:2],
                        start=False, stop=True,
                    )
                    nc.scalar.activation(
                        out=s_cols[:, blk : blk + 1], in_=psy[:],
                        func=AF.Sigmoid, bias=b2c_sb[:, blk : blk + 1], scale=1.0,
                    )

                if img == n_img - 1:
                    # tail: no further pooling to hide behind; emit directly
                    do_mul_stores(img, xb, s_cols, tail=True)
                else:
                    deferred = (img, xb, s_cols)

    _split_multiwait(nc)
    return nc


# ---------------------------------------------------------------------------
# host-side driver
# ---------------------------------------------------------------------------

_CACHED = {}


def _get_nc():
    if "nc" not in _CACHED:
        _CACHED["nc"] = build_nc()
    return _CACHED["nc"]


def kernel(x, w1, b1, w2, b2):

    from concourse.bass_utils import run_bass_kernel_spmd

    x = np.asarray(x, dtype=np.float32)
    assert x.shape == (B, 112, 112, C)
    xr = x.reshape(B, HWs, C)
    w1 = np.ascontiguousarray(w1, dtype=np.float32)
    b1 = np.ascontiguousarray(b1, dtype=np.float32)
    w2 = np.ascontiguousarray(w2, dtype=np.float32)
    b2c = np.ascontiguousarray(
        (2.0 * np.asarray(b2, dtype=np.float32)).reshape(NBLK, P).T
    )
    in_maps = []
    for c in range(N_CORES):
        xc = xr[c * IMG : (c + 1) * IMG]  # [4, 12544, 256]
        xc = np.ascontiguousarray(xc.transpose(0, 2, 1)).astype(np.float16)
        in_maps.append(
            {
                "x": xc.reshape(IMG * C, HWs),
                "w1": w1,
                "b1": b1,
                "w2": w2,
                "b2c": b2c,
            }
        )
    nc = _get_nc()
    res = run_bass_kernel_spmd(nc, in_maps, list(range(N_CORES)))
    out = np.empty((B, C, HWs), dtype=np.float32)
    for c in range(N_CORES):
        out[c * IMG : (c + 1) * IMG] = (
            res.results[c]["y"].reshape(IMG, C, HWs).astype(np.float32)
        )
    return np.ascontiguousarray(out.transpose(0, 2, 1)).reshape(B, 112, 112, C)


# revision 22
# speedup vs baseline: 1.3067x; 1.0803x over previous
"""ChannelAttention (CBAM-style) Trainium2 Bass kernel.

Reference computation (per batch image):
    avg = mean(x, spatial)             # [C]
    mx  = max(x, spatial)              # [C]
    s   = sigmoid(mlp(avg) + mlp(max)) # mlp: relu(p@w1+b1)@w2+b2
    y   = x * s[None, None, :]

Full shapes: x [32, 112, 112, 256] f32.  Data-parallel over batch: each of
the 8 NeuronCores handles 4 images; the tiny MLP weights are replicated.

Per-core structure (4 images, channel-major fp16 [C=2x128p, HW=12544]):
  - Pooling: BOTH stats ride the DVE's 4x-perf-mode tensor_scalar(x*1)
    accumulator: op1=add gives the free-axis sum, op1=max the free-axis
    max (0.26 ns/el) -- one instruction per stat per channel block, no
    fold trees.  13.1us/img on DVE.
  - MLP on PE (fp32 matmuls, column-major) + ACT (Relu/Sigmoid share one
    activation table with Copy, so no table reloads).
  - Scale multiply in-place (fp16 tile rewritten as bf16): DVE
    tensor_scalar_mul for cols [0,M1) at 0.26 ns/el, ACT Copy-with-scale
    for [M1,HW) at 0.83 ns/el, sized so both engines finish together.
    The DVE chunk for image i is emitted after image i+1's pooling so the
    DVE never stalls waiting on sigmoid_i.
  - DMA: quarter-block (3136-col) transfers, 2.42us each in the cost
    model, spread over the three DMA queues (SP and ACT HWDGE, Pool
    SWDGE) so every queue carries ~16.7us/img; loads prefetch one image
    ahead, stores fire per quarter as soon as its multiply chunk lands.
Every track (SP, Pool, ACT, DVE) carries ~16.7us/img; makespan is
4 x 16.7us plus pipeline fill/drain.
"""

import sys

import numpy as np

for _p in ("/opt/trn_rl_repo",):
    if _p not in sys.path:
        sys.path.append(_p)

import concourse.bass as bass
import concourse.tile as tile
from concourse import mybir

B, HWs, C = 32, 112 * 112, 256
HID = 32
N_CORES = 8
IMG = B // N_CORES  # 4 images per core
P = 128
NBLK = C // P  # 2 channel blocks
QU = HWs // 4  # 3136-col DMA quarter
F32 = mybir.dt.float32
F16 = mybir.dt.float16
BF16 = mybir.dt.bfloat16

# multiply column split: DVE does [0, M1), ACT does [M1, HWs) per block
M1 = 5050


def _split_multiwait(nc, max_waits=1):
    """This walrus build rejects >1 sync wait per instruction.  Hoist
    excess waits onto InstNoOp instructions inserted just before, on the
    same engine (same semantics: the sequencer blocks on each in turn)."""
    for f in nc.m.functions:
        for bb in f.blocks:
            new_insts = []
            for ins in bb.instructions:
                si = ins.sync_info
                w = list(si.on_wait) if si and si.on_wait else []
                if len(w) > max_waits:
                    for j, ww in enumerate(w[:-max_waits]):
                        nop = mybir.InstNoOp(
                            name=f"{ins.name}.sw{j}",
                            engine=ins.engine,
                            sync_info=mybir.SyncInfo(on_wait=[ww], on_update=[]),
                        )
                        nc.register_instruction(nop, overwrite=True)
                        new_insts.append(nop)
                    si.on_wait = w[-max_waits:]
                new_insts.append(ins)
            bb.instructions = new_insts
    return nc


def build_nc(n_img=IMG):
    nc = bass.Bass()
    x_d = nc.declare_dram_parameter("x", [n_img * C, HWs], F16, isOutput=False)
    w1_d = nc.declare_dram_parameter("w1", [C, HID], F32, isOutput=False)
    b1_d = nc.declare_dram_parameter("b1", [HID], F32, isOutput=False)
    w2_d = nc.declare_dram_parameter("w2", [HID, C], F32, isOutput=False)
    b2c_d = nc.declare_dram_parameter("b2c", [P, NBLK], F32, isOutput=False)
    y_d = nc.declare_dram_parameter("y", [n_img * C, HWs], BF16, isOutput=True)

    xv = x_d.rearrange("(i b p) s -> i b p s", i=n_img, b=NBLK)
    yv = y_d.rearrange("(i b p) s -> i b p s", i=n_img, b=NBLK)

    AF = mybir.ActivationFunctionType
    OP = mybir.AluOpType

    with tile.TileContext(nc) as tc:
        with (
            tc.tile_pool(name="singles", bufs=1) as singles,
            tc.tile_pool(name="xb", bufs=4) as xb_pool,
            tc.tile_pool(name="small", bufs=3) as small,
            tc.tile_pool(name="ps_small", bufs=4, space="PSUM") as ps_small,
        ):
            # --- replicated constants, declared up front, loaded after the
            # first image's quarters are in flight (MLP needs them ~15us in)
            w1_sb = singles.tile([P, NBLK, HID], F32)
            w2_sb = singles.tile([HID, NBLK, P], F32)
            b1_sb = singles.tile([HID, 1], F32)
            b2c_sb = singles.tile([P, NBLK], F32)

            def load_consts():
                nc.scalar.dma_start(out=w1_sb[:], in_=w1_d.rearrange("(b p) h -> p b h", b=NBLK))
                nc.scalar.dma_start(out=w2_sb[:], in_=w2_d.rearrange("h (b p) -> h b p", b=NBLK))
                nc.scalar.dma_start(out=b1_sb[:], in_=b1_d.rearrange("(p o) -> p o", o=1))
                nc.scalar.dma_start(out=b2c_sb[:], in_=b2c_d[:, :])

            # quarter-granular DMA queue assignment (per image):
            # loads: SP x3, Pool x4, ACT x1; stores: SP x4, Pool x3, ACT x1
            LOADQ = ["sync", "gpsimd", "scalar", "sync",
                     "gpsimd", "sync", "gpsimd", "gpsimd"]
            STOREQ = ["sync", "gpsimd", "sync", "scalar",
                      "gpsimd", "sync", "gpsimd", "sync"]
            q = lambda nm: {"sync": nc.sync, "scalar": nc.scalar,
                            "gpsimd": nc.gpsimd}[nm]

            def issue_loads(img):
                t = [
                    xb_pool.tile([P, HWs], F16, tag=f"xb{blk}", name=f"xb{blk}_{img}")
                    for blk in range(NBLK)
                ]
                if img == 0:
                    # fill: block 0's quarters spread over all three queues so
                    # the first pooling instruction starts ~5us sooner
                    qs = ["sync", "gpsimd", "scalar", "sync",
                          "gpsimd", "scalar", "gpsimd", "sync"]
                else:
                    qs = LOADQ
                for k, (blk, j) in enumerate(
                    [(b, j) for b in range(NBLK) for j in range(4)]
                ):
                    sl = slice(j * QU, (j + 1) * QU)
                    q(qs[k]).dma_start(out=t[blk][:, sl], in_=xv[img, blk][:, sl])
                return t

            def do_mul_stores(img, xb, s_cols, tail=False):
                """Multiply chunks + their stores for `img`.  The DVE chunk
                covers [0, m1); ACT covers [m1, HWs).  On the tail image the
                split leans DVE-ward and muls/stores interleave per block so
                the drain is as short as possible."""
                m1 = 3 * QU if tail else M1
                # tail store queues chosen so each queue gets at most 2 of
                # the final quarters and block 1's land on distinct queues
                tailq = ["sync", "gpsimd", "scalar", "sync",
                         "gpsimd", "scalar", "sync", "gpsimd"]
                for blk in range(NBLK):
                    nc.vector.tensor_scalar_mul(
                        out=xb[blk][:, 0:m1].bitcast(BF16),
                        in0=xb[blk][:, 0:m1],
                        scalar1=s_cols[:, blk : blk + 1],
                    )
                    nc.scalar.activation(
                        out=xb[blk][:, m1:HWs].bitcast(BF16),
                        in_=xb[blk][:, m1:HWs], func=AF.Copy,
                        scale=s_cols[:, blk : blk + 1],
                    )
                    if tail:
                        for j in range(4):
                            sl = slice(j * QU, (j + 1) * QU)
                            q(tailq[blk * 4 + j]).dma_start(
                                out=yv[img, blk][:, sl],
                                in_=xb[blk][:, sl].bitcast(BF16),
                            )
                if not tail:
                    for k, (blk, j) in enumerate(
                        [(b, j) for b in range(NBLK) for j in range(4)]
                    ):
                        sl = slice(j * QU, (j + 1) * QU)
                        q(STOREQ[k]).dma_start(
                            out=yv[img, blk][:, sl], in_=xb[blk][:, sl].bitcast(BF16)
                        )

            xb_next = issue_loads(0)
            deferred = None  # (img, xb, s_cols) awaiting DVE mul + stores

            for img in range(n_img):
                xb = xb_next
                if img + 1 < n_img:
                    xb_next = issue_loads(img + 1)
                if img == 0:
                    load_consts()

                # --- pooling (DVE 4x accumulate: op1=add -> sum, op1=max) ---
                sums = small.tile([P, NBLK], F32, tag="sums")
                pmax = small.tile([P, NBLK], F32, tag="pmax")
                for blk in range(NBLK):
                    if img == 0 and blk == 0:
                        # fill: pool the first block in halves so the DVE
                        # starts ~2.5us earlier; the second chunk's scalar2
                        # operand folds in the first chunk's partial (the
                        # accumulator applies op1 against scalar2).
                        part_s = small.tile([P, 1], F32, tag="part_s")
                        part_m = small.tile([P, 1], F32, tag="part_m")
                        HALF = HWs // 2
                        nc.vector.tensor_scalar(
                            out=xb[0][:, 0:HALF], in0=xb[0][:, 0:HALF],
                            scalar1=1.0, scalar2=0.0, op0=OP.mult, op1=OP.add,
                            accum_out=part_s[:, 0:1],
                        )
                        nc.vector.tensor_scalar(
                            out=xb[0][:, 0:HALF], in0=xb[0][:, 0:HALF],
                            scalar1=1.0, scalar2=None, op0=OP.mult, op1=OP.max,
                            accum_out=part_m[:, 0:1],
                        )
                        nc.vector.tensor_scalar(
                            out=xb[0][:, HALF:HWs], in0=xb[0][:, HALF:HWs],
                            scalar1=1.0, scalar2=0.0, op0=OP.mult, op1=OP.add,
                            accum_out=sums[:, 0:1],
                        )
                        nc.vector.tensor_scalar(
                            out=xb[0][:, HALF:HWs], in0=xb[0][:, HALF:HWs],
                            scalar1=1.0, scalar2=None, op0=OP.mult, op1=OP.max,
                            accum_out=pmax[:, 0:1],
                        )
                        nc.vector.tensor_tensor(
                            out=sums[:, 0:1], in0=sums[:, 0:1],
                            in1=part_s[:, 0:1], op=OP.add,
                        )
                        nc.vector.tensor_tensor(
                            out=pmax[:, 0:1], in0=pmax[:, 0:1],
                            in1=part_m[:, 0:1], op=OP.max,
                        )
                        continue
                    nc.vector.tensor_scalar(
                        out=xb[blk][:, :], in0=xb[blk][:, :],
                        scalar1=1.0, scalar2=0.0, op0=OP.mult, op1=OP.add,
                        accum_out=sums[:, blk : blk + 1],
                    )
                    nc.vector.tensor_scalar(
                        out=xb[blk][:, :], in0=xb[blk][:, :],
                        scalar1=1.0, scalar2=None, op0=OP.mult, op1=OP.max,
                        accum_out=pmax[:, blk : blk + 1],
                    )

                # previous image's deferred DVE mul chunk + all its stores:
                # placed after this image's pooling so the DVE never waits
                # on the (long ready) previous sigmoid.
                if deferred is not None:
                    pimg, pxb, psc = deferred
                    deferred = None
                    do_mul_stores(pimg, pxb, psc)

                # --- MLP (PE + ACT), column-major ---
                h_sb = small.tile([HID, 2], F32, tag="h")
                for j, (pc, sc) in enumerate(((sums, 1.0 / HWs), (pmax, 1.0))):
                    ph = ps_small.tile([HID, 1], F32, tag="ph")
                    nc.tensor.matmul(
                        ph[:], lhsT=w1_sb[:, 0, :], rhs=pc[:, 0:1],
                        start=True, stop=False,
                    )
                    nc.tensor.matmul(
                        ph[:], lhsT=w1_sb[:, 1, :], rhs=pc[:, 1:2],
                        start=False, stop=True,
                    )
                    nc.scalar.activation(
                        out=h_sb[:, j : j + 1], in_=ph[:], func=AF.Relu,
                        bias=b1_sb[:], scale=sc,
                    )
                s_cols = small.tile([P, NBLK], F32, tag="scol")
                for blk in range(NBLK):
                    psy = ps_small.tile([P, 1], F32, tag="psy")
                    nc.tensor.matmul(
                        psy[:], lhsT=w2_sb[:, blk, :], rhs=h_sb[:, 0:1],
                        start=True, stop=False,
                    )
                    nc.tensor.matmul(
                        psy[:], lhsT=w2_sb[:, blk, :], rhs=h_sb[:, 1:2],
                        start=False, stop=True,
                    )
                    nc.scalar.activation(
                        out=s_cols[:, blk : blk + 1], in_=psy[:],
                        func=AF.Sigmoid, bias=b2c_sb[:, blk : blk + 1], scale=1.0,
                    )

                if img == n_img - 1:
                    # tail: no further pooling to hide behind; emit directly
                    do_mul_stores(img, xb, s_cols, tail=True)
                else:
                    deferred = (img, xb, s_cols)

    _split_multiwait(nc)
    return nc


# ---------------------------------------------------------------------------
# host-side driver
# ---------------------------------------------------------------------------

_CACHED = {}


def _get_nc():
    if "nc" not in _CACHED:
        _CACHED["nc"] = build_nc()
    return _CACHED["nc"]


def kernel(x, w1, b1, w2, b2):

    from concourse.bass_utils import run_bass_kernel_spmd

    x = np.asarray(x, dtype=np.float32)
    assert x.shape == (B, 112, 112, C)
    xr = x.reshape(B, HWs, C)
    w1 = np.ascontiguousarray(w1, dtype=np.float32)
    b1 = np.ascontiguousarray(b1, dtype=np.float32)
    w2 = np.ascontiguousarray(w2, dtype=np.float32)
    b2c = np.ascontiguousarray(
        (2.0 * np.asarray(b2, dtype=np.float32)).reshape(NBLK, P).T
    )
    in_maps = []
    for c in range(N_CORES):
        xc = xr[c * IMG : (c + 1) * IMG]  # [4, 12544, 256]
        xc = np.ascontiguousarray(xc.transpose(0, 2, 1)).astype(np.float16)
        in_maps.append(
            {
                "x": xc.reshape(IMG * C, HWs),
                "w1": w1,
                "b1": b1,
                "w2": w2,
                "b2c": b2c,
            }
        )
    nc = _get_nc()
    res = run_bass_kernel_spmd(nc, in_maps, list(range(N_CORES)))
    out = np.empty((B, C, HWs), dtype=np.float32)
    for c in range(N_CORES):
        out[c * IMG : (c + 1) * IMG] = (
            res.results[c]["y"].reshape(IMG, C, HWs).astype(np.float32)
        )
    return np.ascontiguousarray(out.transpose(0, 2, 1)).reshape(B, 112, 112, C)


# revision 28
# speedup vs baseline: 1.3078x; 1.0008x over previous
"""ChannelAttention (CBAM-style) Trainium2 Bass kernel.

Reference computation (per batch image):
    avg = mean(x, spatial)             # [C]
    mx  = max(x, spatial)              # [C]
    s   = sigmoid(mlp(avg) + mlp(max)) # mlp: relu(p@w1+b1)@w2+b2
    y   = x * s[None, None, :]

Full shapes: x [32, 112, 112, 256] f32.  Data-parallel over batch: each of
the 8 NeuronCores handles 4 images; the tiny MLP weights are replicated.

Per-core structure (4 images, channel-major fp16 [C=2x128p, HW=12544]):
  - Pooling: BOTH stats ride the DVE's 4x-perf-mode tensor_scalar(x*1)
    accumulator: op1=add gives the free-axis sum, op1=max the free-axis
    max (0.26 ns/el) -- one instruction per stat per channel block, no
    fold trees.  13.1us/img on DVE.
  - MLP on PE (fp32 matmuls, column-major) + ACT (Relu/Sigmoid share one
    activation table with Copy, so no table reloads).
  - Scale multiply in-place (fp16 tile rewritten as bf16): DVE
    tensor_scalar_mul for cols [0,M1) at 0.26 ns/el, ACT Copy-with-scale
    for [M1,HW) at 0.83 ns/el, sized so both engines finish together.
    The DVE chunk for image i is emitted after image i+1's pooling so the
    DVE never stalls waiting on sigmoid_i.
  - DMA: quarter-block (3136-col) transfers, 2.42us each in the cost
    model, spread over the three DMA queues (SP and ACT HWDGE, Pool
    SWDGE); loads prefetch one image ahead with all four images resident
    (bufs=4), stores fire per quarter as soon as its multiply chunk
    lands; image 0's first block is pooled in halves so the DVE starts
    at ~5.5us.
Every track (SP, Pool, ACT, DVE) carries ~16.5-17us/img; makespan is
4 x ~16.7us plus fill/drain: ~79.2us (baseline 103.5us).

Paths measured far cheaper by the cost model but rejected by the real
compiler/hardware (and therefore not used):
  - DMA accum_op=mult (DRAM->DRAM multiply-in-DMA): walrus
    assertDMACopySupportedCceOp rejects mult.
  - gpsimd tensor ops (Pool-engine elementwise/accum): ISA opcode check
    rejects TENSOR_SCALAR on Pool; Pool is a DMA queue only.
  - PE- or DVE-issued HWDGE DMAs (a 4th/5th DMA queue): NEFF load fails.
  - Spatial-major ("transposed") DMA access patterns: CoreSim's race
    detector and interp shadow memory misparse them as conflicting.
"""

import sys

import numpy as np

for _p in ("/opt/trn_rl_repo",):
    if _p not in sys.path:
        sys.path.append(_p)

import concourse.bass as bass
import concourse.tile as tile
from concourse import mybir

B, HWs, C = 32, 112 * 112, 256
HID = 32
N_CORES = 8
IMG = B // N_CORES  # 4 images per core
P = 128
NBLK = C // P  # 2 channel blocks
QU = HWs // 4  # 3136-col DMA quarter
F32 = mybir.dt.float32
F16 = mybir.dt.float16
BF16 = mybir.dt.bfloat16

# multiply column split: DVE does [0, M1), ACT does [M1, HWs) per block
M1 = 5050


def _split_multiwait(nc, max_waits=1):
    """This walrus build rejects >1 sync wait per instruction.  Hoist
    excess waits onto InstNoOp instructions inserted just before, on the
    same engine (same semantics: the sequencer blocks on each in turn)."""
    for f in nc.m.functions:
        for bb in f.blocks:
            new_insts = []
            for ins in bb.instructions:
                si = ins.sync_info
                w = list(si.on_wait) if si and si.on_wait else []
                if len(w) > max_waits:
                    for j, ww in enumerate(w[:-max_waits]):
                        nop = mybir.InstNoOp(
                            name=f"{ins.name}.sw{j}",
                            engine=ins.engine,
                            sync_info=mybir.SyncInfo(on_wait=[ww], on_update=[]),
                        )
                        nc.register_instruction(nop, overwrite=True)
                        new_insts.append(nop)
                    si.on_wait = w[-max_waits:]
                new_insts.append(ins)
            bb.instructions = new_insts
    return nc


def build_nc(n_img=IMG, loadq=None, storeq=None, fillq=None):
    nc = bass.Bass()
    x_d = nc.declare_dram_parameter("x", [n_img * C, HWs], F16, isOutput=False)
    w1_d = nc.declare_dram_parameter("w1", [C, HID], F32, isOutput=False)
    b1_d = nc.declare_dram_parameter("b1", [HID], F32, isOutput=False)
    w2_d = nc.declare_dram_parameter("w2", [HID, C], F32, isOutput=False)
    b2c_d = nc.declare_dram_parameter("b2c", [P, NBLK], F32, isOutput=False)
    y_d = nc.declare_dram_parameter("y", [n_img * C, HWs], BF16, isOutput=True)

    xv = x_d.rearrange("(i b p) s -> i b p s", i=n_img, b=NBLK)
    yv = y_d.rearrange("(i b p) s -> i b p s", i=n_img, b=NBLK)

    AF = mybir.ActivationFunctionType
    OP = mybir.AluOpType

    with tile.TileContext(nc) as tc:
        with (
            tc.tile_pool(name="singles", bufs=1) as singles,
            tc.tile_pool(name="xb", bufs=4) as xb_pool,
            tc.tile_pool(name="small", bufs=3) as small,
            tc.tile_pool(name="ps_small", bufs=4, space="PSUM") as ps_small,
        ):
            # --- replicated constants, declared up front, loaded after the
            # first image's quarters are in flight (MLP needs them ~15us in)
            w1_sb = singles.tile([P, NBLK, HID], F32)
            w2_sb = singles.tile([HID, NBLK, P], F32)
            b1_sb = singles.tile([HID, 1], F32)
            b2c_sb = singles.tile([P, NBLK], F32)

            def load_consts():
                nc.scalar.dma_start(out=w1_sb[:], in_=w1_d.rearrange("(b p) h -> p b h", b=NBLK))
                nc.scalar.dma_start(out=w2_sb[:], in_=w2_d.rearrange("h (b p) -> h b p", b=NBLK))
                nc.scalar.dma_start(out=b1_sb[:], in_=b1_d.rearrange("(p o) -> p o", o=1))
                nc.scalar.dma_start(out=b2c_sb[:], in_=b2c_d[:, :])

            # quarter-granular DMA queue assignment (per image):
            # loads: SP x3, Pool x4, ACT x1; stores: SP x4, Pool x3, ACT x1
            LOADQ = loadq or ["sync", "gpsimd", "scalar", "sync",
                              "gpsimd", "sync", "gpsimd", "gpsimd"]
            STOREQ = storeq or ["sync", "gpsimd", "sync", "scalar",
                                "gpsimd", "sync", "gpsimd", "sync"]
            FILLQ = fillq or ["sync", "gpsimd", "scalar", "sync",
                              "gpsimd", "scalar", "gpsimd", "sync"]
            q = lambda nm: {"sync": nc.sync, "scalar": nc.scalar,
                            "gpsimd": nc.gpsimd}[nm]

            def issue_loads(img):
                t = [
                    xb_pool.tile([P, HWs], F16, tag=f"xb{blk}", name=f"xb{blk}_{img}")
                    for blk in range(NBLK)
                ]
                # img 0: block 0's quarters spread over all three queues so
                # the first pooling instruction starts ~5us sooner
                qs = FILLQ if img == 0 else LOADQ
                for k, (blk, j) in enumerate(
                    [(b, j) for b in range(NBLK) for j in range(4)]
                ):
                    sl = slice(j * QU, (j + 1) * QU)
                    q(qs[k]).dma_start(out=t[blk][:, sl], in_=xv[img, blk][:, sl])
                return t

            def do_mul_stores(img, xb, s_cols, tail=False):
                """Multiply chunks + their stores for `img`.  The DVE chunk
                covers [0, m1); ACT covers [m1, HWs).  On the tail image the
                split leans DVE-ward and muls/stores interleave per block so
                the drain is as short as possible."""
                if tail:
                    # drain: ACT immediately covers block 1's last quarter,
                    # the DVE sweeps block 0 whole then block 1's first three
                    # quarters; every store fires as soon as its chunk lands.
                    nc.scalar.activation(
                        out=xb[1][:, 3 * QU : HWs].bitcast(BF16),
                        in_=xb[1][:, 3 * QU : HWs], func=AF.Copy,
                        scale=s_cols[:, 1:2],
                    )
                    q("scalar").dma_start(
                        out=yv[img, 1][:, 3 * QU : HWs],
                        in_=xb[1][:, 3 * QU : HWs].bitcast(BF16),
                    )
                    nc.vector.tensor_scalar_mul(
                        out=xb[0][:, :].bitcast(BF16), in0=xb[0][:, :],
                        scalar1=s_cols[:, 0:1],
                    )
                    for j, qn in enumerate(["sync", "gpsimd", "scalar", "sync"]):
                        sl = slice(j * QU, (j + 1) * QU)
                        q(qn).dma_start(
                            out=yv[img, 0][:, sl], in_=xb[0][:, sl].bitcast(BF16)
                        )
                    nc.vector.tensor_scalar_mul(
                        out=xb[1][:, 0 : 3 * QU].bitcast(BF16),
                        in0=xb[1][:, 0 : 3 * QU],
                        scalar1=s_cols[:, 1:2],
                    )
                    for j, qn in enumerate(["gpsimd", "sync", "scalar"]):
                        sl = slice(j * QU, (j + 1) * QU)
                        q(qn).dma_start(
                            out=yv[img, 1][:, sl], in_=xb[1][:, sl].bitcast(BF16)
                        )
                    return
                m1 = M1
                for blk in range(NBLK):
                    nc.vector.tensor_scalar_mul(
                        out=xb[blk][:, 0:m1].bitcast(BF16),
                        in0=xb[blk][:, 0:m1],
                        scalar1=s_cols[:, blk : blk + 1],
                    )
                    nc.scalar.activation(
                        out=xb[blk][:, m1:HWs].bitcast(BF16),
                        in_=xb[blk][:, m1:HWs], func=AF.Copy,
                        scale=s_cols[:, blk : blk + 1],
                    )
                if True:
                    for k, (blk, j) in enumerate(
                        [(b, j) for b in range(NBLK) for j in range(4)]
                    ):
                        sl = slice(j * QU, (j + 1) * QU)
                        q(STOREQ[k]).dma_start(
                            out=yv[img, blk][:, sl], in_=xb[blk][:, sl].bitcast(BF16)
                        )

            xb_next = issue_loads(0)
            deferred = None  # (img, xb, s_cols) awaiting DVE mul + stores

            for img in range(n_img):
                xb = xb_next
                if img + 1 < n_img:
                    xb_next = issue_loads(img + 1)
                if img == 0:
                    load_consts()

                # --- pooling (DVE 4x accumulate: op1=add -> sum, op1=max) ---
                sums = small.tile([P, NBLK], F32, tag="sums")
                pmax = small.tile([P, NBLK], F32, tag="pmax")
                for blk in range(NBLK):
                    if img == 0 and blk == 0:
                        # fill: pool the first block in halves so the DVE
                        # starts ~2.5us earlier; the second chunk's scalar2
                        # operand folds in the first chunk's partial (the
                        # accumulator applies op1 against scalar2).
                        part_s = small.tile([P, 1], F32, tag="part_s")
                        part_m = small.tile([P, 1], F32, tag="part_m")
                        HALF = HWs // 2
                        nc.vector.tensor_scalar(
                            out=xb[0][:, 0:HALF], in0=xb[0][:, 0:HALF],
                            scalar1=1.0, scalar2=0.0, op0=OP.mult, op1=OP.add,
                            accum_out=part_s[:, 0:1],
                        )
                        nc.vector.tensor_scalar(
                            out=xb[0][:, 0:HALF], in0=xb[0][:, 0:HALF],
                            scalar1=1.0, scalar2=None, op0=OP.mult, op1=OP.max,
                            accum_out=part_m[:, 0:1],
                        )
                        nc.vector.tensor_scalar(
                            out=xb[0][:, HALF:HWs], in0=xb[0][:, HALF:HWs],
                            scalar1=1.0, scalar2=0.0, op0=OP.mult, op1=OP.add,
                            accum_out=sums[:, 0:1],
                        )
                        nc.vector.tensor_scalar(
                            out=xb[0][:, HALF:HWs], in0=xb[0][:, HALF:HWs],
                            scalar1=1.0, scalar2=None, op0=OP.mult, op1=OP.max,
                            accum_out=pmax[:, 0:1],
                        )
                        nc.vector.tensor_tensor(
                            out=sums[:, 0:1], in0=sums[:, 0:1],
                            in1=part_s[:, 0:1], op=OP.add,
                        )
                        nc.vector.tensor_tensor(
                            out=pmax[:, 0:1], in0=pmax[:, 0:1],
                            in1=part_m[:, 0:1], op=OP.max,
                        )
                        continue
                    nc.vector.tensor_scalar(
                        out=xb[blk][:, :], in0=xb[blk][:, :],
                        scalar1=1.0, scalar2=0.0, op0=OP.mult, op1=OP.add,
                        accum_out=sums[:, blk : blk + 1],
                    )
                    nc.vector.tensor_scalar(
                        out=xb[blk][:, :], in0=xb[blk][:, :],
                        scalar1=1.0, scalar2=None, op0=OP.mult, op1=OP.max,
                        accum_out=pmax[:, blk : blk + 1],
                    )

                # previous image's deferred DVE mul chunk + all its stores:
                # placed after this image's pooling so the DVE never waits
                # on the (long ready) previous sigmoid.
                if deferred is not None:
                    pimg, pxb, psc = deferred
                    deferred = None
                    do_mul_stores(pimg, pxb, psc)

                # --- MLP (PE + ACT), column-major ---
                h_sb = small.tile([HID, 2], F32, tag="h")
                for j, (pc, sc) in enumerate(((sums, 1.0 / HWs), (pmax, 1.0))):
                    ph = ps_small.tile([HID, 1], F32, tag="ph")
                    nc.tensor.matmul(
                        ph[:], lhsT=w1_sb[:, 0, :], rhs=pc[:, 0:1],
                        start=True, stop=False,
                    )
                    nc.tensor.matmul(
                        ph[:], lhsT=w1_sb[:, 1, :], rhs=pc[:, 1:2],
                        start=False, stop=True,
                    )
                    nc.scalar.activation(
                        out=h_sb[:, j : j + 1], in_=ph[:], func=AF.Relu,
                        bias=b1_sb[:], scale=sc,
                    )
                s_cols = small.tile([P, NBLK], F32, tag="scol")
                for blk in range(NBLK):
                    psy = ps_small.tile([P, 1], F32, tag="psy")
                    nc.tensor.matmul(
                        psy[:], lhsT=w2_sb[:, blk, :], rhs=h_sb[:, 0:1],
                        start=True, stop=False,
                    )
                    nc.tensor.matmul(
                        psy[:], lhsT=w2_sb[:, blk, :], rhs=h_sb[:, 1:2],
                        start=False, stop=True,
                    )
                    nc.scalar.activation(
                        out=s_cols[:, blk : blk + 1], in_=psy[:],
                        func=AF.Sigmoid, bias=b2c_sb[:, blk : blk + 1], scale=1.0,
                    )

                if img == n_img - 1:
                    # tail: no further pooling to hide behind; emit directly
                    do_mul_stores(img, xb, s_cols, tail=True)
                else:
                    deferred = (img, xb, s_cols)

    _split_multiwait(nc)
    return nc


# ---------------------------------------------------------------------------
# host-side driver
# ---------------------------------------------------------------------------

_CACHED = {}


def _get_nc():
    if "nc" not in _CACHED:
        _CACHED["nc"] = build_nc()
    return _CACHED["nc"]


def kernel(x, w1, b1, w2, b2):

    from concourse.bass_utils import run_bass_kernel_spmd

    x = np.asarray(x, dtype=np.float32)
    assert x.shape == (B, 112, 112, C)
    xr = x.reshape(B, HWs, C)
    w1 = np.ascontiguousarray(w1, dtype=np.float32)
    b1 = np.ascontiguousarray(b1, dtype=np.float32)
    w2 = np.ascontiguousarray(w2, dtype=np.float32)
    b2c = np.ascontiguousarray(
        (2.0 * np.asarray(b2, dtype=np.float32)).reshape(NBLK, P).T
    )
    in_maps = []
    for c in range(N_CORES):
        xc = xr[c * IMG : (c + 1) * IMG]  # [4, 12544, 256]
        xc = np.ascontiguousarray(xc.transpose(0, 2, 1)).astype(np.float16)
        in_maps.append(
            {
                "x": xc.reshape(IMG * C, HWs),
                "w1": w1,
                "b1": b1,
                "w2": w2,
                "b2c": b2c,
            }
        )
    nc = _get_nc()
    res = run_bass_kernel_spmd(nc, in_maps, list(range(N_CORES)))
    out = np.empty((B, C, HWs), dtype=np.float32)
    for c in range(N_CORES):
        out[c * IMG : (c + 1) * IMG] = (
            res.results[c]["y"].reshape(IMG, C, HWs).astype(np.float32)
        )
    return np.ascontiguousarray(out.transpose(0, 2, 1)).reshape(B, 112, 112, C)


# revision 31
# speedup vs baseline: 1.3237x; 1.0122x over previous
"""ChannelAttention (CBAM-style) Trainium2 Bass kernel.

Reference computation (per batch image):
    avg = mean(x, spatial)             # [C]
    mx  = max(x, spatial)              # [C]
    s   = sigmoid(mlp(avg) + mlp(max)) # mlp: relu(p@w1+b1)@w2+b2
    y   = x * s[None, None, :]

Full shapes: x [32, 112, 112, 256] f32.  Data-parallel over batch: each of
the 8 NeuronCores handles 4 images; the tiny MLP weights are replicated.

Per-core structure (4 images, channel-major fp16 [C=2x128p, HW=12544]):
  - Pooling: BOTH stats ride the DVE's 4x-perf-mode tensor_scalar(x*1)
    accumulator: op1=add gives the free-axis sum, op1=max the free-axis
    max (0.26 ns/el) -- one instruction per stat per channel block, no
    fold trees.  13.1us/img on DVE.
  - MLP on PE (fp32 matmuls, column-major) + ACT (Relu/Sigmoid share one
    activation table with Copy, so no table reloads).
  - Scale multiply in-place (fp16 tile rewritten as bf16): DVE
    tensor_scalar_mul for cols [0,M1) at 0.26 ns/el, ACT Copy-with-scale
    for [M1,HW) at 0.83 ns/el, sized so both engines finish together.
    The DVE chunk for image i is emitted after image i+1's pooling so the
    DVE never stalls waiting on sigmoid_i.
  - DMA: quarter-block (3136-col) transfers, 2.42us each in the cost
    model, spread over the three DMA queues (SP and ACT HWDGE, Pool
    SWDGE); loads prefetch one image ahead with all four images resident
    (bufs=4), stores fire per quarter as soon as its multiply chunk
    lands; image 0's first block is pooled in halves so the DVE starts
    at ~5.5us.
Every track (SP, Pool, ACT, DVE) carries ~16.5-17us/img; makespan is
4 x ~16.7us plus fill/drain: 78.2us (baseline 103.5us), hardware-
validated at max rel err 1.747e-2 (< the 2e-2 gate; the error floor is
set by fp16-subnormal x elements, identical to the baseline's numerics).
M1 and the queue patterns are empirical optima from simulator scans; the
schedule is insensitive to most queue permutations at this fixed point.

Paths measured far cheaper by the cost model but rejected by the real
compiler/hardware (and therefore not used):
  - DMA accum_op=mult (DRAM->DRAM multiply-in-DMA): walrus
    assertDMACopySupportedCceOp rejects mult.
  - gpsimd tensor ops (Pool-engine elementwise/accum): ISA opcode check
    rejects TENSOR_SCALAR on Pool; Pool is a DMA queue only.
  - PE- or DVE-issued HWDGE DMAs (a 4th/5th DMA queue): NEFF load fails.
  - Spatial-major ("transposed") DMA access patterns: CoreSim's race
    detector and interp shadow memory misparse them as conflicting.
"""

import sys

import numpy as np

for _p in ("/opt/trn_rl_repo",):
    if _p not in sys.path:
        sys.path.append(_p)

import concourse.bass as bass
import concourse.tile as tile
from concourse import mybir

B, HWs, C = 32, 112 * 112, 256
HID = 32
N_CORES = 8
IMG = B // N_CORES  # 4 images per core
P = 128
NBLK = C // P  # 2 channel blocks
QU = HWs // 4  # 3136-col DMA quarter
F32 = mybir.dt.float32
F16 = mybir.dt.float16
BF16 = mybir.dt.bfloat16

# multiply column split: DVE does [0, M1), ACT does [M1, HWs) per block
M1 = 4350


def _split_multiwait(nc, max_waits=1):
    """This walrus build rejects >1 sync wait per instruction.  Hoist
    excess waits onto InstNoOp instructions inserted just before, on the
    same engine (same semantics: the sequencer blocks on each in turn)."""
    for f in nc.m.functions:
        for bb in f.blocks:
            new_insts = []
            for ins in bb.instructions:
                si = ins.sync_info
                w = list(si.on_wait) if si and si.on_wait else []
                if len(w) > max_waits:
                    for j, ww in enumerate(w[:-max_waits]):
                        nop = mybir.InstNoOp(
                            name=f"{ins.name}.sw{j}",
                            engine=ins.engine,
                            sync_info=mybir.SyncInfo(on_wait=[ww], on_update=[]),
                        )
                        nc.register_instruction(nop, overwrite=True)
                        new_insts.append(nop)
                    si.on_wait = w[-max_waits:]
                new_insts.append(ins)
            bb.instructions = new_insts
    return nc


def build_nc(n_img=IMG, loadq=None, storeq=None, fillq=None):
    nc = bass.Bass()
    x_d = nc.declare_dram_parameter("x", [n_img * C, HWs], F16, isOutput=False)
    w1_d = nc.declare_dram_parameter("w1", [C, HID], F32, isOutput=False)
    b1_d = nc.declare_dram_parameter("b1", [HID], F32, isOutput=False)
    w2_d = nc.declare_dram_parameter("w2", [HID, C], F32, isOutput=False)
    b2c_d = nc.declare_dram_parameter("b2c", [P, NBLK], F32, isOutput=False)
    y_d = nc.declare_dram_parameter("y", [n_img * C, HWs], BF16, isOutput=True)

    xv = x_d.rearrange("(i b p) s -> i b p s", i=n_img, b=NBLK)
    yv = y_d.rearrange("(i b p) s -> i b p s", i=n_img, b=NBLK)

    AF = mybir.ActivationFunctionType
    OP = mybir.AluOpType

    with tile.TileContext(nc) as tc:
        with (
            tc.tile_pool(name="singles", bufs=1) as singles,
            tc.tile_pool(name="xb", bufs=4) as xb_pool,
            tc.tile_pool(name="small", bufs=3) as small,
            tc.tile_pool(name="ps_small", bufs=4, space="PSUM") as ps_small,
        ):
            # --- replicated constants, declared up front, loaded after the
            # first image's quarters are in flight (MLP needs them ~15us in)
            w1_sb = singles.tile([P, NBLK, HID], F32)
            w2_sb = singles.tile([HID, NBLK, P], F32)
            b1_sb = singles.tile([HID, 1], F32)
            b2c_sb = singles.tile([P, NBLK], F32)

            def load_consts():
                nc.scalar.dma_start(out=w1_sb[:], in_=w1_d.rearrange("(b p) h -> p b h", b=NBLK))
                nc.scalar.dma_start(out=w2_sb[:], in_=w2_d.rearrange("h (b p) -> h b p", b=NBLK))
                nc.scalar.dma_start(out=b1_sb[:], in_=b1_d.rearrange("(p o) -> p o", o=1))
                nc.scalar.dma_start(out=b2c_sb[:], in_=b2c_d[:, :])

            # quarter-granular DMA queue assignment (per image):
            # loads: SP x3, Pool x4, ACT x1; stores: SP x4, Pool x3, ACT x1
            LOADQ = loadq or ["sync", "gpsimd", "scalar", "sync",
                              "gpsimd", "sync", "gpsimd", "gpsimd"]
            STOREQ = storeq or ["sync", "gpsimd", "sync", "gpsimd",
                                "gpsimd", "sync", "gpsimd", "sync"]
            FILLQ = fillq or ["sync", "gpsimd", "scalar", "sync",
                              "gpsimd", "scalar", "gpsimd", "sync"]
            q = lambda nm: {"sync": nc.sync, "scalar": nc.scalar,
                            "gpsimd": nc.gpsimd}[nm]

            def issue_loads(img):
                t = [
                    xb_pool.tile([P, HWs], F16, tag=f"xb{blk}", name=f"xb{blk}_{img}")
                    for blk in range(NBLK)
                ]
                # img 0: block 0's quarters spread over all three queues so
                # the first pooling instruction starts ~5us sooner
                qs = FILLQ if img == 0 else LOADQ
                for k, (blk, j) in enumerate(
                    [(b, j) for b in range(NBLK) for j in range(4)]
                ):
                    sl = slice(j * QU, (j + 1) * QU)
                    q(qs[k]).dma_start(out=t[blk][:, sl], in_=xv[img, blk][:, sl])
                return t

            def do_mul_stores(img, xb, s_cols, tail=False):
                """Multiply chunks + their stores for `img`.  The DVE chunk
                covers [0, m1); ACT covers [m1, HWs).  On the tail image the
                split leans DVE-ward and muls/stores interleave per block so
                the drain is as short as possible."""
                if tail:
                    # drain: ACT immediately covers block 1's last quarter,
                    # the DVE sweeps block 0 whole then block 1's first three
                    # quarters; every store fires as soon as its chunk lands.
                    nc.scalar.activation(
                        out=xb[1][:, 3 * QU : HWs].bitcast(BF16),
                        in_=xb[1][:, 3 * QU : HWs], func=AF.Copy,
                        scale=s_cols[:, 1:2],
                    )
                    q("scalar").dma_start(
                        out=yv[img, 1][:, 3 * QU : HWs],
                        in_=xb[1][:, 3 * QU : HWs].bitcast(BF16),
                    )
                    nc.vector.tensor_scalar_mul(
                        out=xb[0][:, :].bitcast(BF16), in0=xb[0][:, :],
                        scalar1=s_cols[:, 0:1],
                    )
                    for j, qn in enumerate(["sync", "gpsimd", "scalar", "sync"]):
                        sl = slice(j * QU, (j + 1) * QU)
                        q(qn).dma_start(
                            out=yv[img, 0][:, sl], in_=xb[0][:, sl].bitcast(BF16)
                        )
                    nc.vector.tensor_scalar_mul(
                        out=xb[1][:, 0 : 3 * QU].bitcast(BF16),
                        in0=xb[1][:, 0 : 3 * QU],
                        scalar1=s_cols[:, 1:2],
                    )
                    for j, qn in enumerate(["gpsimd", "sync", "scalar"]):
                        sl = slice(j * QU, (j + 1) * QU)
                        q(qn).dma_start(
                            out=yv[img, 1][:, sl], in_=xb[1][:, sl].bitcast(BF16)
                        )
                    return
                m1 = M1
                for blk in range(NBLK):
                    nc.vector.tensor_scalar_mul(
                        out=xb[blk][:, 0:m1].bitcast(BF16),
                        in0=xb[blk][:, 0:m1],
                        scalar1=s_cols[:, blk : blk + 1],
                    )
                    nc.scalar.activation(
                        out=xb[blk][:, m1:HWs].bitcast(BF16),
                        in_=xb[blk][:, m1:HWs], func=AF.Copy,
                        scale=s_cols[:, blk : blk + 1],
                    )
                if True:
                    for k, (blk, j) in enumerate(
                        [(b, j) for b in range(NBLK) for j in range(4)]
                    ):
                        sl = slice(j * QU, (j + 1) * QU)
                        q(STOREQ[k]).dma_start(
                            out=yv[img, blk][:, sl], in_=xb[blk][:, sl].bitcast(BF16)
                        )

            xb_next = issue_loads(0)
            deferred = None  # (img, xb, s_cols) awaiting DVE mul + stores

            for img in range(n_img):
                xb = xb_next
                if img + 1 < n_img:
                    xb_next = issue_loads(img + 1)
                if img == 0:
                    load_consts()

                # --- pooling (DVE 4x accumulate: op1=add -> sum, op1=max) ---
                sums = small.tile([P, NBLK], F32, tag="sums")
                pmax = small.tile([P, NBLK], F32, tag="pmax")
                for blk in range(NBLK):
                    if img == 0 and blk == 0:
                        # fill: pool the first block in halves so the DVE
                        # starts ~2.5us earlier; the second chunk's scalar2
                        # operand folds in the first chunk's partial (the
                        # accumulator applies op1 against scalar2).
                        part_s = small.tile([P, 1], F32, tag="part_s")
                        part_m = small.tile([P, 1], F32, tag="part_m")
                        HALF = HWs // 2
                        nc.vector.tensor_scalar(
                            out=xb[0][:, 0:HALF], in0=xb[0][:, 0:HALF],
                            scalar1=1.0, scalar2=0.0, op0=OP.mult, op1=OP.add,
                            accum_out=part_s[:, 0:1],
                        )
                        nc.vector.tensor_scalar(
                            out=xb[0][:, 0:HALF], in0=xb[0][:, 0:HALF],
                            scalar1=1.0, scalar2=None, op0=OP.mult, op1=OP.max,
                            accum_out=part_m[:, 0:1],
                        )
                        nc.vector.tensor_scalar(
                            out=xb[0][:, HALF:HWs], in0=xb[0][:, HALF:HWs],
                            scalar1=1.0, scalar2=0.0, op0=OP.mult, op1=OP.add,
                            accum_out=sums[:, 0:1],
                        )
                        nc.vector.tensor_scalar(
                            out=xb[0][:, HALF:HWs], in0=xb[0][:, HALF:HWs],
                            scalar1=1.0, scalar2=None, op0=OP.mult, op1=OP.max,
                            accum_out=pmax[:, 0:1],
                        )
                        nc.vector.tensor_tensor(
                            out=sums[:, 0:1], in0=sums[:, 0:1],
                            in1=part_s[:, 0:1], op=OP.add,
                        )
                        nc.vector.tensor_tensor(
                            out=pmax[:, 0:1], in0=pmax[:, 0:1],
                            in1=part_m[:, 0:1], op=OP.max,
                        )
                        continue
                    nc.vector.tensor_scalar(
                        out=xb[blk][:, :], in0=xb[blk][:, :],
                        scalar1=1.0, scalar2=0.0, op0=OP.mult, op1=OP.add,
                        accum_out=sums[:, blk : blk + 1],
                    )
                    nc.vector.tensor_scalar(
                        out=xb[blk][:, :], in0=xb[blk][:, :],
                        scalar1=1.0, scalar2=None, op0=OP.mult, op1=OP.max,
                        accum_out=pmax[:, blk : blk + 1],
                    )

                # previous image's deferred DVE mul chunk + all its stores:
                # placed after this image's pooling so the DVE never waits
                # on the (long ready) previous sigmoid.
                if deferred is not None:
                    pimg, pxb, psc = deferred
                    deferred = None
                    do_mul_stores(pimg, pxb, psc)

                # --- MLP (PE + ACT), column-major ---
                h_sb = small.tile([HID, 2], F32, tag="h")
                for j, (pc, sc) in enumerate(((sums, 1.0 / HWs), (pmax, 1.0))):
                    ph = ps_small.tile([HID, 1], F32, tag="ph")
                    nc.tensor.matmul(
                        ph[:], lhsT=w1_sb[:, 0, :], rhs=pc[:, 0:1],
                        start=True, stop=False,
                    )
                    nc.tensor.matmul(
                        ph[:], lhsT=w1_sb[:, 1, :], rhs=pc[:, 1:2],
                        start=False, stop=True,
                    )
                    nc.scalar.activation(
                        out=h_sb[:, j : j + 1], in_=ph[:], func=AF.Relu,
                        bias=b1_sb[:], scale=sc,
                    )
                s_cols = small.tile([P, NBLK], F32, tag="scol")
                for blk in range(NBLK):
                    psy = ps_small.tile([P, 1], F32, tag="psy")
                    nc.tensor.matmul(
                        psy[:], lhsT=w2_sb[:, blk, :], rhs=h_sb[:, 0:1],
                        start=True, stop=False,
                    )
                    nc.tensor.matmul(
                        psy[:], lhsT=w2_sb[:, blk, :], rhs=h_sb[:, 1:2],
                        start=False, stop=True,
                    )
                    nc.scalar.activation(
                        out=s_cols[:, blk : blk + 1], in_=psy[:],
                        func=AF.Sigmoid, bias=b2c_sb[:, blk : blk + 1], scale=1.0,
                    )

                if img == n_img - 1:
                    # tail: no further pooling to hide behind; emit directly
                    do_mul_stores(img, xb, s_cols, tail=True)
                else:
                    deferred = (img, xb, s_cols)

    _split_multiwait(nc)
    return nc


# ---------------------------------------------------------------------------
# host-side driver
# ---------------------------------------------------------------------------

_CACHED = {}


def _get_nc():
    if "nc" not in _CACHED:
        _CACHED["nc"] = build_nc()
    return _CACHED["nc"]


def kernel(x, w1, b1, w2, b2):

    from concourse.bass_utils import run_bass_kernel_spmd

    x = np.asarray(x, dtype=np.float32)
    assert x.shape == (B, 112, 112, C)
    xr = x.reshape(B, HWs, C)
    w1 = np.ascontiguousarray(w1, dtype=np.float32)
    b1 = np.ascontiguousarray(b1, dtype=np.float32)
    w2 = np.ascontiguousarray(w2, dtype=np.float32)
    b2c = np.ascontiguousarray(
        (2.0 * np.asarray(b2, dtype=np.float32)).reshape(NBLK, P).T
    )
    in_maps = []
    for c in range(N_CORES):
        xc = xr[c * IMG : (c + 1) * IMG]  # [4, 12544, 256]
        xc = np.ascontiguousarray(xc.transpose(0, 2, 1)).astype(np.float16)
        in_maps.append(
            {
                "x": xc.reshape(IMG * C, HWs),
                "w1": w1,
                "b1": b1,
                "w2": w2,
                "b2c": b2c,
            }
        )
    nc = _get_nc()
    res = run_bass_kernel_spmd(nc, in_maps, list(range(N_CORES)))
    out = np.empty((B, C, HWs), dtype=np.float32)
    for c in range(N_CORES):
        out[c * IMG : (c + 1) * IMG] = (
            res.results[c]["y"].reshape(IMG, C, HWs).astype(np.float32)
        )
    return np.ascontiguousarray(out.transpose(0, 2, 1)).reshape(B, 112, 112, C)


# revision 35
# speedup vs baseline: 1.3354x; 1.0088x over previous
"""ChannelAttention (CBAM-style) Trainium2 Bass kernel.

Reference computation (per batch image):
    avg = mean(x, spatial)             # [C]
    mx  = max(x, spatial)              # [C]
    s   = sigmoid(mlp(avg) + mlp(max)) # mlp: relu(p@w1+b1)@w2+b2
    y   = x * s[None, None, :]

Full shapes: x [32, 112, 112, 256] f32.  Data-parallel over batch: each of
the 8 NeuronCores handles 4 images; the tiny MLP weights are replicated.

Per-core structure (4 images, channel-major fp16 [C=2x128p, HW=12544]):
  - Pooling: BOTH stats ride the DVE's 4x-perf-mode tensor_scalar(x*1)
    accumulator: op1=add gives the free-axis sum, op1=max the free-axis
    max (0.26 ns/el) -- one instruction per stat per channel block, no
    fold trees.  13.1us/img on DVE.
  - MLP on PE (fp32 matmuls, column-major) + ACT (Relu/Sigmoid share one
    activation table with Copy, so no table reloads).
  - Scale multiply in-place (fp16 tile rewritten as bf16): DVE
    tensor_scalar_mul for cols [0,M1) at 0.26 ns/el, ACT Copy-with-scale
    for [M1,HW) at 0.83 ns/el, sized so both engines finish together.
    The DVE chunk for image i is emitted after image i+1's pooling so the
    DVE never stalls waiting on sigmoid_i.
  - DMA: quarter-block (3136-col) transfers, 2.42us each in the cost
    model, spread over the three DMA queues (SP and ACT HWDGE, Pool
    SWDGE); loads prefetch one image ahead with all four images resident
    (bufs=4), stores fire per quarter as soon as its multiply chunk
    lands; image 0's first block is pooled in halves so the DVE starts
    at ~5.5us.
Every track (SP, Pool, ACT, DVE) carries ~16.5-17us/img; makespan is
4 x ~16.7us plus fill/drain: 77.5us (baseline 103.5us), hardware-
validated at max rel err 1.747e-2 (< the 2e-2 gate; the error floor is
set by fp16-subnormal x elements, identical to the baseline's numerics).
M1 and the queue patterns are empirical optima from simulator scans; the
schedule is insensitive to most queue permutations at this fixed point.

Paths measured far cheaper by the cost model but rejected by the real
compiler/hardware (and therefore not used):
  - DMA accum_op=mult (DRAM->DRAM multiply-in-DMA): walrus
    assertDMACopySupportedCceOp rejects mult.
  - gpsimd tensor ops (Pool-engine elementwise/accum): ISA opcode check
    rejects TENSOR_SCALAR on Pool; Pool is a DMA queue only.
  - PE- or DVE-issued HWDGE DMAs (a 4th/5th DMA queue): NEFF load fails.
  - Spatial-major ("transposed") DMA access patterns: CoreSim's race
    detector and interp shadow memory misparse them as conflicting.
"""

import sys

import numpy as np

for _p in ("/opt/trn_rl_repo",):
    if _p not in sys.path:
        sys.path.append(_p)

import concourse.bass as bass
import concourse.tile as tile
from concourse import mybir

B, HWs, C = 32, 112 * 112, 256
HID = 32
N_CORES = 8
IMG = B // N_CORES  # 4 images per core
P = 128
NBLK = C // P  # 2 channel blocks
QU = HWs // 4  # 3136-col DMA quarter
F32 = mybir.dt.float32
F16 = mybir.dt.float16
BF16 = mybir.dt.bfloat16

# multiply column split: DVE does [0, M1), ACT does [M1, HWs) per block
M1 = 4050


def _split_multiwait(nc, max_waits=1):
    """This walrus build rejects >1 sync wait per instruction.  Hoist
    excess waits onto InstNoOp instructions inserted just before, on the
    same engine (same semantics: the sequencer blocks on each in turn)."""
    for f in nc.m.functions:
        for bb in f.blocks:
            new_insts = []
            for ins in bb.instructions:
                si = ins.sync_info
                w = list(si.on_wait) if si and si.on_wait else []
                if len(w) > max_waits:
                    for j, ww in enumerate(w[:-max_waits]):
                        nop = mybir.InstNoOp(
                            name=f"{ins.name}.sw{j}",
                            engine=ins.engine,
                            sync_info=mybir.SyncInfo(on_wait=[ww], on_update=[]),
                        )
                        nc.register_instruction(nop, overwrite=True)
                        new_insts.append(nop)
                    si.on_wait = w[-max_waits:]
                new_insts.append(ins)
            bb.instructions = new_insts
    return nc


def build_nc(n_img=IMG, loadq=None, storeq=None, fillq=None, m1_img2=None):
    nc = bass.Bass()
    x_d = nc.declare_dram_parameter("x", [n_img * C, HWs], F16, isOutput=False)
    w1_d = nc.declare_dram_parameter("w1", [C, HID], F32, isOutput=False)
    b1_d = nc.declare_dram_parameter("b1", [HID], F32, isOutput=False)
    w2_d = nc.declare_dram_parameter("w2", [HID, C], F32, isOutput=False)
    b2c_d = nc.declare_dram_parameter("b2c", [P, NBLK], F32, isOutput=False)
    y_d = nc.declare_dram_parameter("y", [n_img * C, HWs], BF16, isOutput=True)

    xv = x_d.rearrange("(i b p) s -> i b p s", i=n_img, b=NBLK)
    yv = y_d.rearrange("(i b p) s -> i b p s", i=n_img, b=NBLK)

    AF = mybir.ActivationFunctionType
    OP = mybir.AluOpType

    with tile.TileContext(nc) as tc:
        with (
            tc.tile_pool(name="singles", bufs=1) as singles,
            tc.tile_pool(name="xb", bufs=4) as xb_pool,
            tc.tile_pool(name="small", bufs=3) as small,
            tc.tile_pool(name="ps_small", bufs=4, space="PSUM") as ps_small,
        ):
            # --- replicated constants, declared up front, loaded after the
            # first image's quarters are in flight (MLP needs them ~15us in)
            w1_sb = singles.tile([P, NBLK, HID], F32)
            w2_sb = singles.tile([HID, NBLK, P], F32)
            b1_sb = singles.tile([HID, 1], F32)
            b2c_sb = singles.tile([P, NBLK], F32)

            def load_consts():
                nc.scalar.dma_start(out=w1_sb[:], in_=w1_d.rearrange("(b p) h -> p b h", b=NBLK))
                nc.scalar.dma_start(out=w2_sb[:], in_=w2_d.rearrange("h (b p) -> h b p", b=NBLK))
                nc.scalar.dma_start(out=b1_sb[:], in_=b1_d.rearrange("(p o) -> p o", o=1))
                nc.scalar.dma_start(out=b2c_sb[:], in_=b2c_d[:, :])

            # quarter-granular DMA queue assignment (per image):
            # loads: SP x3, Pool x4, ACT x1; stores: SP x4, Pool x3, ACT x1
            LOADQ = loadq or ["sync", "gpsimd", "scalar", "sync",
                              "gpsimd", "sync", "gpsimd", "gpsimd"]
            STOREQ = storeq or ["sync", "gpsimd", "sync", "gpsimd",
                                "gpsimd", "sync", "gpsimd", "sync"]
            FILLQ = fillq or ["sync", "gpsimd", "scalar", "sync",
                              "gpsimd", "scalar", "gpsimd", "sync"]
            q = lambda nm: {"sync": nc.sync, "scalar": nc.scalar,
                            "gpsimd": nc.gpsimd}[nm]

            def issue_loads(img):
                t = [
                    xb_pool.tile([P, HWs], F16, tag=f"xb{blk}", name=f"xb{blk}_{img}")
                    for blk in range(NBLK)
                ]
                # img 0: block 0's quarters spread over all three queues so
                # the first pooling instruction starts ~5us sooner
                qs = FILLQ if img == 0 else LOADQ
                for k, (blk, j) in enumerate(
                    [(b, j) for b in range(NBLK) for j in range(4)]
                ):
                    sl = slice(j * QU, (j + 1) * QU)
                    q(qs[k]).dma_start(out=t[blk][:, sl], in_=xv[img, blk][:, sl])
                return t

            def do_mul_stores(img, xb, s_cols, tail=False):
                """Multiply chunks + their stores for `img`.  The DVE chunk
                covers [0, m1); ACT covers [m1, HWs).  On the tail image the
                split leans DVE-ward and muls/stores interleave per block so
                the drain is as short as possible."""
                if tail:
                    # drain: ACT immediately covers block 1's last quarter,
                    # the DVE sweeps block 0 whole then block 1's first three
                    # quarters; every store fires as soon as its chunk lands.
                    nc.scalar.activation(
                        out=xb[1][:, 3 * QU : HWs].bitcast(BF16),
                        in_=xb[1][:, 3 * QU : HWs], func=AF.Copy,
                        scale=s_cols[:, 1:2],
                    )
                    q("scalar").dma_start(
                        out=yv[img, 1][:, 3 * QU : HWs],
                        in_=xb[1][:, 3 * QU : HWs].bitcast(BF16),
                    )
                    nc.vector.tensor_scalar_mul(
                        out=xb[0][:, :].bitcast(BF16), in0=xb[0][:, :],
                        scalar1=s_cols[:, 0:1],
                    )
                    for j, qn in enumerate(["sync", "gpsimd", "scalar", "scalar"]):
                        sl = slice(j * QU, (j + 1) * QU)
                        q(qn).dma_start(
                            out=yv[img, 0][:, sl], in_=xb[0][:, sl].bitcast(BF16)
                        )
                    nc.vector.tensor_scalar_mul(
                        out=xb[1][:, 0 : 3 * QU].bitcast(BF16),
                        in0=xb[1][:, 0 : 3 * QU],
                        scalar1=s_cols[:, 1:2],
                    )
                    for j, qn in enumerate(["gpsimd", "sync", "scalar"]):
                        sl = slice(j * QU, (j + 1) * QU)
                        q(qn).dma_start(
                            out=yv[img, 1][:, sl], in_=xb[1][:, sl].bitcast(BF16)
                        )
                    return
                m1 = m1_img2 if (m1_img2 and img == n_img - 2) else M1
                for blk in range(NBLK):
                    nc.vector.tensor_scalar_mul(
                        out=xb[blk][:, 0:m1].bitcast(BF16),
                        in0=xb[blk][:, 0:m1],
                        scalar1=s_cols[:, blk : blk + 1],
                    )
                    nc.scalar.activation(
                        out=xb[blk][:, m1:HWs].bitcast(BF16),
                        in_=xb[blk][:, m1:HWs], func=AF.Copy,
                        scale=s_cols[:, blk : blk + 1],
                    )
                if True:
                    for k, (blk, j) in enumerate(
                        [(b, j) for b in range(NBLK) for j in range(4)]
                    ):
                        sl = slice(j * QU, (j + 1) * QU)
                        q(STOREQ[k]).dma_start(
                            out=yv[img, blk][:, sl], in_=xb[blk][:, sl].bitcast(BF16)
                        )

            xb_next = issue_loads(0)
            deferred = None  # (img, xb, s_cols) awaiting DVE mul + stores

            for img in range(n_img):
                xb = xb_next
                if img + 1 < n_img:
                    xb_next = issue_loads(img + 1)
                if img == 0:
                    load_consts()

                # --- pooling (DVE 4x accumulate: op1=add -> sum, op1=max) ---
                sums = small.tile([P, NBLK], F32, tag="sums")
                pmax = small.tile([P, NBLK], F32, tag="pmax")
                for blk in range(NBLK):
                    if img == 0 and blk == 0:
                        # fill: pool the first block in halves so the DVE
                        # starts ~2.5us earlier; the second chunk's scalar2
                        # operand folds in the first chunk's partial (the
                        # accumulator applies op1 against scalar2).
                        part_s = small.tile([P, 1], F32, tag="part_s")
                        part_m = small.tile([P, 1], F32, tag="part_m")
                        HALF = HWs // 2
                        nc.vector.tensor_scalar(
                            out=xb[0][:, 0:HALF], in0=xb[0][:, 0:HALF],
                            scalar1=1.0, scalar2=0.0, op0=OP.mult, op1=OP.add,
                            accum_out=part_s[:, 0:1],
                        )
                        nc.vector.tensor_scalar(
                            out=xb[0][:, 0:HALF], in0=xb[0][:, 0:HALF],
                            scalar1=1.0, scalar2=None, op0=OP.mult, op1=OP.max,
                            accum_out=part_m[:, 0:1],
                        )
                        nc.vector.tensor_scalar(
                            out=xb[0][:, HALF:HWs], in0=xb[0][:, HALF:HWs],
                            scalar1=1.0, scalar2=0.0, op0=OP.mult, op1=OP.add,
                            accum_out=sums[:, 0:1],
                        )
                        nc.vector.tensor_scalar(
                            out=xb[0][:, HALF:HWs], in0=xb[0][:, HALF:HWs],
                            scalar1=1.0, scalar2=None, op0=OP.mult, op1=OP.max,
                            accum_out=pmax[:, 0:1],
                        )
                        nc.vector.tensor_tensor(
                            out=sums[:, 0:1], in0=sums[:, 0:1],
                            in1=part_s[:, 0:1], op=OP.add,
                        )
                        nc.vector.tensor_tensor(
                            out=pmax[:, 0:1], in0=pmax[:, 0:1],
                            in1=part_m[:, 0:1], op=OP.max,
                        )
                        continue
                    nc.vector.tensor_scalar(
                        out=xb[blk][:, :], in0=xb[blk][:, :],
                        scalar1=1.0, scalar2=0.0, op0=OP.mult, op1=OP.add,
                        accum_out=sums[:, blk : blk + 1],
                    )
                    nc.vector.tensor_scalar(
                        out=xb[blk][:, :], in0=xb[blk][:, :],
                        scalar1=1.0, scalar2=None, op0=OP.mult, op1=OP.max,
                        accum_out=pmax[:, blk : blk + 1],
                    )

                # previous image's deferred DVE mul chunk + all its stores:
                # placed after this image's pooling so the DVE never waits
                # on the (long ready) previous sigmoid.
                if deferred is not None:
                    pimg, pxb, psc = deferred
                    deferred = None
                    do_mul_stores(pimg, pxb, psc)

                # --- MLP (PE + ACT), column-major ---
                h_sb = small.tile([HID, 2], F32, tag="h")
                for j, (pc, sc) in enumerate(((sums, 1.0 / HWs), (pmax, 1.0))):
                    ph = ps_small.tile([HID, 1], F32, tag="ph")
                    nc.tensor.matmul(
                        ph[:], lhsT=w1_sb[:, 0, :], rhs=pc[:, 0:1],
                        start=True, stop=False,
                    )
                    nc.tensor.matmul(
                        ph[:], lhsT=w1_sb[:, 1, :], rhs=pc[:, 1:2],
                        start=False, stop=True,
                    )
                    nc.scalar.activation(
                        out=h_sb[:, j : j + 1], in_=ph[:], func=AF.Relu,
                        bias=b1_sb[:], scale=sc,
                    )
                s_cols = small.tile([P, NBLK], F32, tag="scol")
                for blk in range(NBLK):
                    psy = ps_small.tile([P, 1], F32, tag="psy")
                    nc.tensor.matmul(
                        psy[:], lhsT=w2_sb[:, blk, :], rhs=h_sb[:, 0:1],
                        start=True, stop=False,
                    )
                    nc.tensor.matmul(
                        psy[:], lhsT=w2_sb[:, blk, :], rhs=h_sb[:, 1:2],
                        start=False, stop=True,
                    )
                    nc.scalar.activation(
                        out=s_cols[:, blk : blk + 1], in_=psy[:],
                        func=AF.Sigmoid, bias=b2c_sb[:, blk : blk + 1], scale=1.0,
                    )

                if img == n_img - 1:
                    # tail: no further pooling to hide behind; emit directly
                    do_mul_stores(img, xb, s_cols, tail=True)
                else:
                    deferred = (img, xb, s_cols)

    _split_multiwait(nc)
    return nc


# ---------------------------------------------------------------------------
# host-side driver
# ---------------------------------------------------------------------------

_CACHED = {}


def _get_nc():
    if "nc" not in _CACHED:
        _CACHED["nc"] = build_nc()
    return _CACHED["nc"]


def kernel(x, w1, b1, w2, b2):

    from concourse.bass_utils import run_bass_kernel_spmd

    x = np.asarray(x, dtype=np.float32)
    assert x.shape == (B, 112, 112, C)
    xr = x.reshape(B, HWs, C)
    w1 = np.ascontiguousarray(w1, dtype=np.float32)
    b1 = np.ascontiguousarray(b1, dtype=np.float32)
    w2 = np.ascontiguousarray(w2, dtype=np.float32)
    b2c = np.ascontiguousarray(
        (2.0 * np.asarray(b2, dtype=np.float32)).reshape(NBLK, P).T
    )
    in_maps = []
    for c in range(N_CORES):
        xc = xr[c * IMG : (c + 1) * IMG]  # [4, 12544, 256]
        xc = np.ascontiguousarray(xc.transpose(0, 2, 1)).astype(np.float16)
        in_maps.append(
            {
                "x": xc.reshape(IMG * C, HWs),
                "w1": w1,
                "b1": b1,
                "w2": w2,
                "b2c": b2c,
            }
        )
    nc = _get_nc()
    res = run_bass_kernel_spmd(nc, in_maps, list(range(N_CORES)))
    out = np.empty((B, C, HWs), dtype=np.float32)
    for c in range(N_CORES):
        out[c * IMG : (c + 1) * IMG] = (
            res.results[c]["y"].reshape(IMG, C, HWs).astype(np.float32)
        )
    return np.ascontiguousarray(out.transpose(0, 2, 1)).reshape(B, 112, 112, C)
